# revision 19
# baseline (speedup 1.0000x reference)
"""Trainium2 Bass kernel for nn_Channel2D2 (gnn_message_passing).

Computation: x [32, 2, 24, L] -> out [32, 2, 6, 5, L] where each of the 30
output nodes is the mean of 3 of the 24 input nodes (fixed triangle table).

Strategy (pure data parallel over batch, 8 cores):
  - Each core gets 4 batches = 8 (b, c) slices of [24, L].
  - The neighbor-average is a tiny matmul: out[30, L] = W[30, 24] @ x[24, L]
    with W holding three 1/3 entries per row. We pack 4 slices per matmul
    group: lhsT = block-diag W^T [K=96, M=120], rhs = x tile [96, N<=512],
    PSUM out [120, N]. Weights are an inline constant; 1/3 scale folded in.
  - PSUM is evicted to SBUF with copies alternating between the Scalar and
    Vector engines, then DMA'd out in multi-MiB transfers. The kernel is
    memory-bound: ~54 MiB of HBM traffic per core at ~285 GB/s practical
    per-core bandwidth under all-8-core load.
"""

import numpy as np

TRI = np.array([
    (0, 3, 4), (0, 1, 4), (1, 4, 5), (1, 2, 5), (2, 5, 6),
    (3, 4, 7), (4, 7, 8), (4, 5, 8), (5, 8, 9), (5, 6, 9),
    (7, 10, 11), (7, 8, 11), (8, 11, 12), (8, 9, 12), (9, 12, 13),
    (10, 11, 14), (11, 14, 15), (11, 12, 15), (12, 15, 16), (12, 13, 16),
    (14, 17, 18), (14, 15, 18), (15, 18, 19), (15, 16, 19), (16, 19, 20),
    (17, 18, 21), (18, 21, 22), (18, 19, 22), (19, 22, 23), (19, 20, 23),
], dtype=np.int32)

B, C, NODES, L = 32, 2, 24, 32768
N_OUT = 30                      # output nodes (6*5 grid)
N_CORES = 8
B_PER_CORE = B // N_CORES       # 4
SLICES = B_PER_CORE * C         # 8 (b, c) slices per core
GROUP = 4                       # slices packed into one matmul
N_GROUPS = SLICES // GROUP      # 2
K = GROUP * NODES               # 96  (contraction dim, partitions)
M = GROUP * N_OUT               # 120 (PSUM partitions)
MM_N = 512                      # fp32 matmul free-dim limit (1 PSUM bank)
EV_N = 2048                     # eviction granularity (4 PSUM banks)
CHUNK = 4096                    # L columns per outer iteration
IN_BUFS = 4                     # input tile slots (keeps read DMAs queued)
OUT_BUFS = 6                    # output tile slots (keeps write DMAs queued)


def _build_w() -> np.ndarray:
    """Block-diagonal lhsT [K, M]: W[s*24 + node, s*30 + out] = 1/3."""
    w = np.zeros((K, M), np.float32)
    for s in range(GROUP):
        for o in range(N_OUT):
            for n in TRI[o]:
                w[s * NODES + int(n), s * N_OUT + o] = np.float32(1.0 / 3.0)
    return w


def _hoist_matmul_waits(nc) -> None:
    """Move excess semaphore waits onto single-wait NoOps before the owner.

    Walrus codegen bakes sync waits into per-instruction ISA structs with
    very few wait slots ("Too many sync wait commands" otherwise): the
    self-loading fp32 Matmult's LDWEIGHTS struct fits none-safely, NoOp and
    DMACopy fit one. So: Matmult keeps zero waits, everything else keeps at
    most one; the rest are hoisted to one-wait NoOps on the same engine.
    Engine-queue FIFO order preserves semantics.
    """
    import concourse.mybir as mybir

    for func in nc.m.functions:
        for blk in func.blocks:
            new_insts = []
            for inst in blk.instructions:
                si = inst.sync_info
                keep = 0 if isinstance(inst, mybir.InstMatmult) else 1
                if (
                    si is not None
                    and si.on_wait
                    and len(si.on_wait) > keep
                    and getattr(inst, "engine", None) is not None
                ):
                    waits = list(si.on_wait)
                    for w in waits[keep:]:
                        nop = mybir.InstNoOp(
                            name=nc.get_next_instruction_name(),
                            ins=[], outs=[])
                        nop.engine = inst.engine
                        nop.sync_info = mybir.SyncInfo(
                            on_wait=[w], on_update=[])
                        new_insts.append(nop)
                    inst.sync_info = mybir.SyncInfo(
                        on_wait=waits[:keep], on_update=list(si.on_update))
                new_insts.append(inst)
            blk.instructions[:] = new_insts


def _emit_pass(nc, pools, x_ap, y_ap, w_tile, wdt, *, chunk, in_split,
               ev_state):
    """Emit one full pass (all groups, all chunks) of the pipeline.

    in_split: number of sub-DMAs the input chunk load is split into
      (1 = one [K, chunk] DMA; 4 = four [K, chunk/4] DMAs so the PE can
      start earlier and its idle gaps stay below the HAM window).
    """
    import concourse.mybir as mybir

    inp, outp, psum = pools
    sub = chunk // in_split
    assert sub % EV_N == 0
    for g in range(N_GROUPS):
        for c0 in range(0, x_ap.shape[1], chunk):
            ot = outp.tile([M, chunk], mybir.dt.float32, tag="ot")
            for s0 in range(0, chunk, sub):
                # separate sub-tiles: each matmul group depends only on its
                # own sub-DMA, so PE idle gaps stay below the HAM window
                it = inp.tile([K, sub], wdt, tag="it")
                nc.sync.dma_start(
                    out=it,
                    in_=x_ap[g * K:(g + 1) * K,
                             c0 + s0:c0 + s0 + sub].bitcast(wdt))
                for j0 in range(0, sub, EV_N):
                    pt = psum.tile([M, EV_N], mybir.dt.float32, tag="pt")
                    for k0 in range(0, EV_N, MM_N):
                        nc.tensor.matmul(
                            out=pt[:, k0:k0 + MM_N],
                            lhsT=w_tile,
                            rhs=it[:, j0 + k0:j0 + k0 + MM_N],
                            start=True, stop=True)
                    if ev_state[0] % 2 == 0:
                        nc.scalar.copy(
                            out=ot[:, s0 + j0:s0 + j0 + EV_N], in_=pt)
                    else:
                        nc.vector.tensor_copy(
                            out=ot[:, s0 + j0:s0 + j0 + EV_N], in_=pt)
                    ev_state[0] += 1
            nc.scalar.dma_start(
                out=y_ap[g * M:(g + 1) * M, c0:c0 + chunk], in_=ot)


def build_nc(l_total: int = L, chunk: int = CHUNK, repeat: int = 1,
             hoist: bool = True, mm_dtype: str = "float32",
             in_bufs: int = IN_BUFS, in_split: int = 1,
             out_bufs: int = OUT_BUFS):
    """Build the per-core Bass module.

    Input  x: [SLICES*NODES, l_total] f32  (slice-major rows)
    Output y: [SLICES*N_OUT, l_total] f32
    """
    import concourse.bass as bass
    import concourse.mybir as mybir
    from concourse.tile import TileContext

    assert l_total % chunk == 0 and chunk % EV_N == 0

    nc = bass.Bass(name="channel2d2")
    wdt = getattr(mybir.dt, mm_dtype)
    x = nc.dram_tensor("x", [SLICES * NODES, l_total], mybir.dt.float32,
                       kind="ExternalInput")
    y = nc.dram_tensor("y", [SLICES * N_OUT, l_total], mybir.dt.float32,
                       kind="ExternalOutput")
    w_dram = nc.inline_tensor(_build_w(), name="w")

    with TileContext(nc) as tc:
        with (
            tc.tile_pool(name="wpool", bufs=1) as wpool,
            tc.tile_pool(name="inp", bufs=in_bufs) as inp,
            tc.tile_pool(name="outp", bufs=out_bufs) as outp,
            tc.tile_pool(name="psum", bufs=2, space="PSUM") as psum,
        ):
            w_tile = wpool.tile([K, M], wdt)
            nc.sync.dma_start(out=w_tile, in_=w_dram[:].bitcast(wdt))
            ev_state = [0]
            for _rep in range(repeat):
                _emit_pass(nc, (inp, outp, psum), x[:], y[:], w_tile, wdt,
                           chunk=chunk, in_split=in_split, ev_state=ev_state)
    if hoist:
        _hoist_matmul_waits(nc)
    return nc


def build_timing_loop_nc(reps: int, chunk: int = CHUNK,
                         mm_dtype: str = "float32", in_bufs: int = IN_BUFS,
                         in_split: int = 1, unroll: int = 1,
                         out_bufs: int = OUT_BUFS):
    """Timing-only variant: the work pass repeats via a hardware For_i loop
    with all data in on-device Internal DRAM (zero-filled once) and dummy
    [1,1] external I/O, so the ~100 ms/call axon overhead is amortized and
    (T(R2)-T(R1))/(R2-R1) isolates on-device execution time. The loop
    back-edge costs ~2-4 us, a slight over-estimate of steady state.
    """
    import concourse.bass as bass
    import concourse.mybir as mybir
    from concourse.tile import TileContext

    nc = bass.Bass(name="channel2d2_timing_loop")
    nc.dram_tensor("x", [1, 1], mybir.dt.float32, kind="ExternalInput")
    dummy_out = nc.dram_tensor("y", [1, 1], mybir.dt.float32,
                               kind="ExternalOutput")
    wdt = getattr(mybir.dt, mm_dtype)
    xs = nc.dram_tensor("xs", [SLICES * NODES, L], mybir.dt.float32)
    ys = nc.dram_tensor("ys", [SLICES * N_OUT, L], mybir.dt.float32)
    w_dram = nc.inline_tensor(_build_w(), name="w")

    with TileContext(nc) as tc:
        with (
            tc.tile_pool(name="wpool", bufs=1) as wpool,
            tc.tile_pool(name="inp", bufs=in_bufs) as inp,
            tc.tile_pool(name="outp", bufs=out_bufs) as outp,
            tc.tile_pool(name="psum", bufs=2, space="PSUM") as psum,
        ):
            w_tile = wpool.tile([K, M], wdt)
            nc.sync.dma_start(out=w_tile, in_=w_dram[:].bitcast(wdt))

            zt = wpool.tile([128, 2048], mybir.dt.float32)
            nc.vector.memset(zt, 0.0)
            for r0 in (0, 64):
                for c0 in range(0, L, 2048):
                    nc.sync.dma_start(
                        out=xs[r0:r0 + 128, c0:c0 + 2048], in_=zt)

            assert reps % unroll == 0
            ev_state = [0]
            with tc.For_i(0, reps // unroll, 1):
                for _u in range(unroll):
                    _emit_pass(nc, (inp, outp, psum), xs[:], ys[:], w_tile,
                               wdt, chunk=chunk, in_split=in_split,
                               ev_state=ev_state)
            nc.sync.dma_start(out=dummy_out[:],
                              in_=w_tile[0:1, 0:1].bitcast(mybir.dt.float32))
    _hoist_matmul_waits(nc)
    return nc


def shard_inputs(x: np.ndarray) -> list[dict]:
    """Full x [B, C, NODES, L] -> per-core in_maps (batch-sharded)."""
    x = np.ascontiguousarray(np.asarray(x, dtype=np.float32))
    assert x.shape == (B, C, NODES, x.shape[-1])
    l_total = x.shape[-1]
    return [
        {"x": x[i * B_PER_CORE:(i + 1) * B_PER_CORE].reshape(
            SLICES * NODES, l_total)}
        for i in range(N_CORES)
    ]


def unshard_outputs(ys: list[np.ndarray]) -> np.ndarray:
    """Per-core y [SLICES*N_OUT, L] list -> full out [B, C, 6, 5, L]."""
    l_total = ys[0].shape[-1]
    out = np.concatenate(
        [yc.reshape(B_PER_CORE, C, N_OUT, l_total) for yc in ys], axis=0)
    return out.reshape(B, C, 6, 5, l_total)


_NC = None


def kernel(x, b=None, l=None, **_unused) -> np.ndarray:
    global _NC
    from concourse import bass_utils

    if _NC is None:
        _NC = build_nc()
    in_maps = shard_inputs(x)
    res = bass_utils.run_bass_kernel_spmd(
        _NC, in_maps, core_ids=list(range(N_CORES)))
    return unshard_outputs([r["y"] for r in res.results])
